# revision 18
# baseline (speedup 1.0000x reference)
"""AdaptiveLayerNorm Trainium2 kernel (8-core SPMD, data-parallel over tokens).

out = sigmoid(LN_w(s) @ W_s.T + b_s) * LN(a) + LN_w(s) @ W_nb.T

Sharding: tokens (B*N = 32768) split evenly across 8 cores; weights replicated.
No collectives needed.

v2 design (engine-balanced, ramp-optimized):
- bf16 on-chip; LN stats fp32; output fp32 DMA'd straight from PSUM.
- fine-grained groups [1,1,2,4,4...] with s-before-a DMA order and weights
  loaded after the first groups' data, so PE starts ~2us in and never sees
  a multi-us DMA gap (HAM stays warm).
- stats trickled per-tile with a 2-group lead: no bursts in the strict-FIFO
  engine queues.
- epilogue: a_n via 4x-mode tensor_scalar (DVE), gate-multiply t2=a_n*g on
  the otherwise-idle GpSimd, and the final "skip + t2" runs on the PE as an
  identity matmul accumulating into the skip PSUM region; the result leaves
  PSUM by DMA (fp32), so DVE never touches PSUM in the epilogue.
- a-stats: sum via tensor_scalar+accum_out (DVE), sum-of-squares via ACT
  Square+accum; s-stats via bn_stats (DVE).
"""

import sys

sys.path.insert(0, "/opt/trn_rl_repo")

import numpy as np
import ml_dtypes

# Problem constants (hardcoded per harness contract)
B, N, CA, CS = 4, 8192, 768, 384
NCORES = 8
TOK = B * N                    # 32768
TPC = TOK // NCORES            # 4096 tokens per core
P = 128                        # partitions / tokens per tile
NTILES = TPC // P              # 32
EPS = 1e-5

GROUPS = [1, 1, 2, 4, 4, 4, 4, 4, 4, 4]   # sum = 32
assert sum(GROUPS) == NTILES
NG = len(GROUPS)
GRP = max(GROUPS)
G_FIRST = [sum(GROUPS[:g]) for g in range(NG)]

_BUILD_CACHE = {}


def _build_graph():
    import concourse.bass as bass
    import concourse.tile as tile
    from concourse import bacc, mybir

    dt = mybir.dt
    AF = mybir.ActivationFunctionType
    OP = mybir.AluOpType

    nc = bacc.Bacc(
        "TRN2",
        target_bir_lowering=False,
        debug=False,
        num_devices=NCORES,
    )

    a_d = nc.dram_tensor("a", [TPC, CA], dt.bfloat16, kind="ExternalInput").ap()
    s_d = nc.dram_tensor("s", [TPC, CS], dt.bfloat16, kind="ExternalInput").ap()
    # wcat = concat([W_s*ln_w, W_nb*ln_w], axis=0).T -> [CS, 2*CA], bf16
    w_d = nc.dram_tensor("wcat", [CS, 2 * CA], dt.bfloat16, kind="ExternalInput").ap()
    # bias row padded to 1024 with zeros so the start=True bias matmuls cover
    # full PSUM banks (bank-granular pending-zero semantics)
    br_d = nc.dram_tensor("brow", [1, 1024], dt.bfloat16, kind="ExternalInput").ap()
    on_d = nc.dram_tensor("ones1", [1, P], dt.bfloat16, kind="ExternalInput").ap()
    id_d = nc.dram_tensor("ident", [P, P], dt.bfloat16, kind="ExternalInput").ap()
    out_d = nc.dram_tensor("out", [TPC, CA], dt.bfloat16, kind="ExternalOutput").ap()

    KC = CS // P  # 3 contraction chunks

    def grp_of(j):
        for g in range(NG):
            if G_FIRST[g] <= j < G_FIRST[g] + GROUPS[g]:
                return g, j - G_FIRST[g]
        raise ValueError(j)

    with tile.TileContext(nc) as tc:
        from contextlib import ExitStack

        with ExitStack() as ctx:
            const = ctx.enter_context(tc.tile_pool(name="const", bufs=1))
            io = ctx.enter_context(tc.tile_pool(name="io", bufs=5))
            scr = ctx.enter_context(tc.tile_pool(name="scr", bufs=2))
            wp = ctx.enter_context(tc.tile_pool(name="wp", bufs=3))
            stat = ctx.enter_context(tc.tile_pool(name="stat", bufs=4))
            pst = ctx.enter_context(tc.tile_pool(name="pst", bufs=2, space="PSUM"))
            pgk_pool = ctx.enter_context(tc.tile_pool(name="pgk", bufs=2, space="PSUM"))

            # ---- tiny constants first ----
            br_sb = const.tile([1, 1024], dt.bfloat16)
            nc.sync.dma_start(out=br_sb[:], in_=br_d[:, :])
            on_sb = const.tile([1, P], dt.bfloat16)
            nc.sync.dma_start(out=on_sb[:], in_=on_d[:, :])
            id_sb = const.tile([P, P], dt.bfloat16)
            nc.sync.dma_start(out=id_sb[:], in_=id_d[:, :])

            # ---- group input tiles (ring-buffered via tags) ----
            a_g = [None] * NG
            s_g = [None] * NG

            def load_group(g):
                n = GROUPS[g]
                g0 = G_FIRST[g] * P
                s_g[g] = io.tile([P, n, CS], dt.bfloat16, name=f"s_g{g}",
                                 tag="s_g", padded_shape=[P, GRP, CS])
                nc.sync.dma_start(
                    out=s_g[g][:],
                    in_=s_d[g0: g0 + n * P, :].rearrange("(q p) c -> p q c", p=P),
                )
                a_g[g] = io.tile([P, n, CA], dt.bfloat16, name=f"a_g{g}",
                                 tag="a_g", padded_shape=[P, GRP, CA])
                nc.sync.dma_start(
                    out=a_g[g][:],
                    in_=a_d[g0: g0 + n * P, :].rearrange("(q p) c -> p q c", p=P),
                )

            # groups 0 and 1 load before the weights; weights next; rest stream.
            load_group(0)
            load_group(1)
            w_sb = const.tile([P, KC, 2 * CA], dt.bfloat16)
            nc.sync.dma_start(out=w_sb[:, 0, :], in_=w_d[0:P, :])
            load_group(2)
            load_group(3)
            for k in range(1, KC):
                nc.sync.dma_start(out=w_sb[:, k, :], in_=w_d[k * P: (k + 1) * P, :])

            # ---- per-group stat tiles ----
            st6 = [None] * NG
            mv = [None] * NG
            inv_s = [None] * NG
            asum = [None] * NG
            assq = [None] * NG
            mu_a = [None] * NG
            y_a = [None] * NG

            def emit_tile_stats(jj):
                """bn_stats for s, sum/sumsq for a, for global tile jj."""
                g, i = grp_of(jj)
                n = GROUPS[g]
                if st6[g] is None:
                    st6[g] = stat.tile([P, n, 6], dt.float32, name=f"st6_{g}",
                                       tag="st6", padded_shape=[P, GRP, 6])
                    asum[g] = stat.tile([P, n], dt.float32, name=f"asum{g}",
                                        tag="asum", padded_shape=[P, GRP])
                    assq[g] = stat.tile([P, n], dt.float32, name=f"assq{g}",
                                        tag="assq", padded_shape=[P, GRP])
                s_t = s_g[g][:, i, :]
                a_t = a_g[g][:, i, :]
                nc.vector.bn_stats(st6[g][:, i, :], s_t[:])
                ascr = scr.tile([P, CA], dt.bfloat16, name="ascr", tag="ascr")
                nc.vector.tensor_scalar(
                    out=ascr[:], in0=a_t[:], scalar1=1.0, scalar2=0.0,
                    op0=OP.mult, op1=OP.add, accum_out=asum[g][:, i: i + 1],
                )
                sscr = scr.tile([P, CA], dt.bfloat16, name="sscr", tag="sscr")
                nc.scalar.activation(
                    out=sscr[:], in_=a_t[:], func=AF.Square,
                    accum_out=assq[g][:, i: i + 1],
                )

            def newton_rsqrt(dst, ve, n_col):
                """dst = 1/sqrt(ve), ve ~ 1.0. One seeded Newton iteration x2."""
                y0 = dst
                nc.vector.tensor_scalar(
                    out=y0[:], in0=ve[:], scalar1=-0.5, scalar2=1.5,
                    op0=OP.mult, op1=OP.add,
                )
                u = stat.tile([P, n_col], dt.float32, name="newt_u", tag="newt_u",
                              padded_shape=[P, GRP])
                nc.vector.tensor_tensor(out=u[:], in0=y0[:], in1=y0[:], op=OP.mult)
                nc.vector.tensor_tensor(out=u[:], in0=u[:], in1=ve[:], op=OP.mult)
                nc.vector.tensor_scalar(
                    out=u[:], in0=u[:], scalar1=-0.5, scalar2=1.5,
                    op0=OP.mult, op1=OP.add,
                )
                nc.vector.tensor_tensor(out=y0[:], in0=y0[:], in1=u[:], op=OP.mult)

            def emit_group_smalls(g):
                n = GROUPS[g]
                mv[g] = stat.tile([P, n, 2], dt.float32, name=f"mv{g}", tag="mv",
                                  padded_shape=[P, GRP, 2])
                for i in range(n):
                    nc.vector.bn_aggr(mv[g][:, i, :], st6[g][:, i, :])
                # s: inv_s = rsqrt(var + eps)
                inv_s[g] = stat.tile([P, n], dt.float32, name=f"invs{g}", tag="invs",
                                     padded_shape=[P, GRP])
                ve = stat.tile([P, n], dt.float32, name="ve_s", tag="ve_s",
                               padded_shape=[P, GRP])
                nc.vector.tensor_scalar(
                    out=ve[:], in0=mv[g][:, :, 1], scalar1=EPS, scalar2=None,
                    op0=OP.add,
                )
                newton_rsqrt(inv_s[g], ve, n)
                # a: mu_a, var_a from asum/assq; y_a = rsqrt(var + eps)
                mu_a[g] = stat.tile([P, n], dt.float32, name=f"mua{g}", tag="mua",
                                    padded_shape=[P, GRP])
                nc.vector.tensor_scalar(
                    out=mu_a[g][:], in0=asum[g][:], scalar1=1.0 / CA, scalar2=None,
                    op0=OP.mult,
                )
                mu2 = stat.tile([P, n], dt.float32, name="mu2", tag="mu2",
                                padded_shape=[P, GRP])
                nc.vector.tensor_tensor(
                    out=mu2[:], in0=mu_a[g][:], in1=mu_a[g][:], op=OP.mult
                )
                va = stat.tile([P, n], dt.float32, name="va", tag="va",
                               padded_shape=[P, GRP])
                nc.vector.scalar_tensor_tensor(
                    out=va[:], in0=assq[g][:], scalar=1.0 / CA, in1=mu2[:],
                    op0=OP.mult, op1=OP.subtract,
                )
                nc.vector.tensor_scalar(
                    out=va[:], in0=va[:], scalar1=EPS, scalar2=None, op0=OP.add,
                )
                y_a[g] = stat.tile([P, n], dt.float32, name=f"ya{g}", tag="ya",
                                   padded_shape=[P, GRP])
                newton_rsqrt(y_a[g], va, n)

            # ---- prologue stats: tiles 0..3 (groups 0..2) primed upfront ----
            for g in (0, 1, 2):
                for i in range(GROUPS[g]):
                    emit_tile_stats(G_FIRST[g] + i)
                emit_group_smalls(g)

            # ---- s-pipeline stage tiles ----
            s_hat_t = [None] * NTILES
            sT_t = [None] * NTILES
            a_n_t = [None] * NTILES
            g_t = [None] * NTILES
            t2_t = [None] * NTILES
            pgk_t = [None] * NTILES
            psT_t = [None] * NTILES

            def emit_s_stage(j):
                """s_hat + PE transposes + copy to SBUF for tile j."""
                g, i = grp_of(j)
                s_t = s_g[g][:, i, :]
                sh = wp.tile([P, CS], dt.bfloat16, name="s_hat", tag="s_hat")
                nc.vector.tensor_scalar(
                    out=sh[:], in0=s_t[:],
                    scalar1=mv[g][:, i, 0:1], scalar2=inv_s[g][:, i: i + 1],
                    op0=OP.subtract, op1=OP.mult,
                )
                s_hat_t[j] = sh
                psT = pst.tile([P, KC, P], dt.bfloat16, name="psT", tag="psT")
                for k in range(KC):
                    nc.tensor.transpose(
                        psT[:, k, :], sh[:, k * P: (k + 1) * P], id_sb[:]
                    )
                psT_t[j] = psT

            def emit_s_copy(j):
                sT = wp.tile([P, KC, P], dt.bfloat16, name="sT", tag="sT")
                nc.scalar.copy(out=sT[:], in_=psT_t[j][:])
                sT_t[j] = sT

            def emit_mains(j):
                """bias + main matmuls for tile j into a fresh pgk buffer."""
                pgk = pgk_pool.tile([P, 3 * 512], dt.float32, name="pgk", tag="pgk")
                pgk_t[j] = pgk
                sT = sT_t[j]
                # bias: start=True clears banks 0 and 1 entirely (bias row is
                # zero-padded over cols 768:1024)
                nc.tensor.matmul(
                    pgk[:, 0:512], lhsT=on_sb[:, :], rhs=br_sb[:, 0:512],
                    start=True, stop=False, skip_group_check=True,
                )
                nc.tensor.matmul(
                    pgk[:, 512:1024], lhsT=on_sb[:, :], rhs=br_sb[:, 512:1024],
                    start=True, stop=False, skip_group_check=True,
                )
                # mains: 3 N-chunks of 512 per k; cols 0:768 gate, 768:1536 skip
                for k in range(KC):
                    for nn in range(3):
                        nsl = slice(nn * 512, (nn + 1) * 512)
                        nc.tensor.matmul(
                            pgk[:, nsl],
                            lhsT=sT[:, k, :],
                            rhs=w_sb[:, k, nsl],
                            start=(k == 0 and nn == 2),
                            stop=(k == KC - 1),
                            skip_group_check=True,
                        )

            def emit_deferred(j):
                """sigmoid + a_n + t2 + PE-add + out DMA for tile j."""
                g, i = grp_of(j)
                a_t = a_g[g][:, i, :]
                pgk = pgk_t[j]
                gt = wp.tile([P, CA], dt.bfloat16, name="g", tag="g")
                nc.scalar.activation(out=gt[:], in_=pgk[:, 0:CA], func=AF.Sigmoid)
                g_t[j] = gt
                an = wp.tile([P, CA], dt.bfloat16, name="a_n", tag="a_n")
                nc.vector.tensor_scalar(
                    out=an[:], in0=a_t[:],
                    scalar1=mu_a[g][:, i: i + 1], scalar2=y_a[g][:, i: i + 1],
                    op0=OP.subtract, op1=OP.mult,
                )
                a_n_t[j] = an
                t2 = wp.tile([P, CA], dt.bfloat16, name="t2", tag="t2")
                nc.gpsimd.tensor_tensor(out=t2[:], in0=an[:], in1=gt[:], op=OP.mult)
                t2_t[j] = t2
                o_t = wp.tile([P, CA], dt.bfloat16, name="o_t", tag="o_t")
                nc.vector.tensor_tensor(
                    out=o_t[:], in0=t2[:], in1=pgk[:, 768:1536], op=OP.add
                )
                r0 = j * P
                nc.sync.dma_start(out=out_d[r0: r0 + P, :], in_=o_t[:])

            # ---- prologue priming of the s-pipeline for tile 0 ----
            emit_s_stage(0)
            emit_s_copy(0)

            # ---- main loop ----
            STAT_LEAD = 4  # stats for tile j+4 are emitted during tile j
            for g in range(NG):
                if g >= 2 and g + 2 < NG:
                    load_group(g + 2)
                for i in range(GROUPS[g]):
                    j = G_FIRST[g] + i
                    # ---- phase B for tile j ----
                    if j > 0:
                        emit_deferred(j - 1)
                    # trickled stats with a flat 4-tile lead
                    jj = j + STAT_LEAD
                    if jj < NTILES:
                        emit_tile_stats(jj)
                        gj, ij = grp_of(jj)
                        if ij == GROUPS[gj] - 1:
                            emit_group_smalls(gj)
                    if j + 1 < NTILES:
                        emit_s_stage(j + 1)
                    emit_mains(j)
                    if j + 1 < NTILES:
                        emit_s_copy(j + 1)

            emit_deferred(NTILES - 1)

    nc.compile()
    return nc


def _get_graph():
    if "nc" not in _BUILD_CACHE:
        _BUILD_CACHE["nc"] = _build_graph()
    return _BUILD_CACHE["nc"]


def _host_prep(a, s, ln_s_w, W_s, b_s, W_nb):
    """Shard inputs and prepare derived weights."""
    bf16 = ml_dtypes.bfloat16
    a2 = np.ascontiguousarray(a.reshape(TOK, CA)).astype(bf16)
    s2 = np.ascontiguousarray(s.reshape(TOK, CS)).astype(bf16)

    wg = (W_s * ln_s_w[None, :]).astype(np.float32)      # [CA, CS]
    wk = (W_nb * ln_s_w[None, :]).astype(np.float32)     # [CA, CS]
    wcat = np.concatenate([wg, wk], axis=0)              # [2CA, CS]
    wcatT = np.ascontiguousarray(wcat.T).astype(bf16)    # [CS, 2CA]
    brow = np.zeros((1, 1024), dtype=bf16)
    brow[0, :CA] = b_s.astype(np.float32).astype(bf16)
    ones1 = np.ones((1, P), dtype=bf16)
    ident = np.eye(P, dtype=bf16)

    in_maps = []
    for c in range(NCORES):
        in_maps.append(
            {
                "a": np.ascontiguousarray(a2[c * TPC: (c + 1) * TPC]),
                "s": np.ascontiguousarray(s2[c * TPC: (c + 1) * TPC]),
                "wcat": wcatT,
                "brow": brow,
                "ones1": ones1,
                "ident": ident,
            }
        )
    return in_maps


def _install_ntff_hook():
    """Register the axon NTFF profile hook that the container's antenv stub lacks."""
    import types
    import antenv

    if "antenv.axon_hooks" not in sys.modules:
        mod = types.ModuleType("antenv.axon_hooks")
        mod._hook = None

        def set_axon_ntff_profile_hook(h):
            mod._hook = h

        def get_axon_ntff_profile_hook():
            return mod._hook

        mod.set_axon_ntff_profile_hook = set_axon_ntff_profile_hook
        mod.get_axon_ntff_profile_hook = get_axon_ntff_profile_hook
        sys.modules["antenv.axon_hooks"] = mod
        antenv.axon_hooks = mod

    hooks = sys.modules["antenv.axon_hooks"]
    if hooks._hook is None:
        from trn_agent_boot.trn_boot import _ntff_profile_via_ctypes

        hooks.set_axon_ntff_profile_hook(
            _ntff_profile_via_ctypes("/opt/axon/libaxon_pjrt.so")
        )

    # upload_artifacts needs external bucket access; stub it out.
    from concourse import bass_utils

    bass_utils.upload_artifacts = lambda tmpdir: f"local:{tmpdir}"


def run(inputs, trace=False):
    """Run on 8 NeuronCores. Returns (out_full [B,N,CA] f32, exec_time_ns|None)."""
    from concourse.bass_utils import run_bass_kernel_spmd

    if trace:
        _install_ntff_hook()
    nc = _get_graph()
    in_maps = _host_prep(**inputs)
    res = run_bass_kernel_spmd(
        nc, in_maps, core_ids=list(range(NCORES)), trace=trace
    )
    outs = [np.asarray(res.results[c]["out"], dtype=np.float32) for c in range(NCORES)]
    full = np.concatenate(outs, axis=0).reshape(B, N, CA)
    return full, res.exec_time_ns


def kernel(**inputs):
    out, _ = run(inputs, trace=False)
    return out


# revision 21
# speedup vs baseline: 1.3738x; 1.3738x over previous
"""AdaptiveLayerNorm Trainium2 kernel (8-core SPMD, data-parallel over tokens).

out = sigmoid(LN_w(s) @ W_s.T + b_s) * LN(a) + LN_w(s) @ W_nb.T

Sharding: tokens (B*N = 32768) split evenly across 8 cores; weights replicated.
No collectives needed.

v2 design (engine-balanced, ramp-optimized):
- bf16 on-chip; LN stats fp32; output fp32 DMA'd straight from PSUM.
- fine-grained groups [1,1,2,4,4...] with s-before-a DMA order and weights
  loaded after the first groups' data, so PE starts ~2us in and never sees
  a multi-us DMA gap (HAM stays warm).
- stats trickled per-tile with a 2-group lead: no bursts in the strict-FIFO
  engine queues.
- epilogue: a_n via 4x-mode tensor_scalar (DVE), gate-multiply t2=a_n*g on
  the otherwise-idle GpSimd, and the final "skip + t2" runs on the PE as an
  identity matmul accumulating into the skip PSUM region; the result leaves
  PSUM by DMA (fp32), so DVE never touches PSUM in the epilogue.
- a-stats: sum via tensor_scalar+accum_out (DVE), sum-of-squares via ACT
  Square+accum; s-stats via bn_stats (DVE).
"""

import sys

sys.path.insert(0, "/opt/trn_rl_repo")

import numpy as np
import ml_dtypes

# Problem constants (hardcoded per harness contract)
B, N, CA, CS = 4, 8192, 768, 384
NCORES = 8
TOK = B * N                    # 32768
TPC = TOK // NCORES            # 4096 tokens per core
P = 128                        # partitions / tokens per tile
NTILES = TPC // P              # 32
EPS = 1e-5

GROUPS = [1, 1, 2, 4, 4, 4, 4, 4, 4, 4]   # sum = 32
assert sum(GROUPS) == NTILES
NG = len(GROUPS)
GRP = max(GROUPS)
G_FIRST = [sum(GROUPS[:g]) for g in range(NG)]

_BUILD_CACHE = {}


def _build_graph():
    import concourse.bass as bass
    import concourse.tile as tile
    from concourse import bacc, mybir

    dt = mybir.dt
    AF = mybir.ActivationFunctionType
    OP = mybir.AluOpType

    nc = bacc.Bacc(
        "TRN2",
        target_bir_lowering=False,
        debug=False,
        num_devices=NCORES,
    )

    a_d = nc.dram_tensor("a", [TPC, CA], dt.bfloat16, kind="ExternalInput").ap()
    s_d = nc.dram_tensor("s", [TPC, CS], dt.bfloat16, kind="ExternalInput").ap()
    # wcat = concat([W_s*ln_w, W_nb*ln_w], axis=0).T -> [CS, 2*CA], bf16
    w_d = nc.dram_tensor("wcat", [CS, 2 * CA], dt.bfloat16, kind="ExternalInput").ap()
    # bias row padded to 1024 with zeros so the start=True bias matmuls cover
    # full PSUM banks (bank-granular pending-zero semantics)
    br_d = nc.dram_tensor("brow", [1, 1024], dt.bfloat16, kind="ExternalInput").ap()
    on_d = nc.dram_tensor("ones1", [1, P], dt.bfloat16, kind="ExternalInput").ap()
    id_d = nc.dram_tensor("ident", [P, P], dt.bfloat16, kind="ExternalInput").ap()
    out_d = nc.dram_tensor("out", [TPC, CA], dt.bfloat16, kind="ExternalOutput").ap()

    KC = CS // P  # 3 contraction chunks

    def grp_of(j):
        for g in range(NG):
            if G_FIRST[g] <= j < G_FIRST[g] + GROUPS[g]:
                return g, j - G_FIRST[g]
        raise ValueError(j)

    with tile.TileContext(nc) as tc:
        from contextlib import ExitStack

        with ExitStack() as ctx:
            const = ctx.enter_context(tc.tile_pool(name="const", bufs=1))
            io = ctx.enter_context(tc.tile_pool(name="io", bufs=5))
            scr = ctx.enter_context(tc.tile_pool(name="scr", bufs=2))
            wp = ctx.enter_context(tc.tile_pool(name="wp", bufs=3))
            stat = ctx.enter_context(tc.tile_pool(name="stat", bufs=4))
            pst = ctx.enter_context(tc.tile_pool(name="pst", bufs=2, space="PSUM"))
            pgk_pool = ctx.enter_context(tc.tile_pool(name="pgk", bufs=2, space="PSUM"))

            # ---- tiny constants first ----
            br_sb = const.tile([1, 1024], dt.bfloat16)
            nc.sync.dma_start(out=br_sb[:], in_=br_d[:, :])
            on_sb = const.tile([1, P], dt.bfloat16)
            nc.sync.dma_start(out=on_sb[:], in_=on_d[:, :])
            id_sb = const.tile([P, P], dt.bfloat16)
            nc.sync.dma_start(out=id_sb[:], in_=id_d[:, :])

            # ---- group input tiles (ring-buffered via tags) ----
            a_g = [None] * NG
            s_g = [None] * NG

            def load_group(g):
                n = GROUPS[g]
                g0 = G_FIRST[g] * P
                s_g[g] = io.tile([P, n, CS], dt.bfloat16, name=f"s_g{g}",
                                 tag="s_g", padded_shape=[P, GRP, CS])
                nc.sync.dma_start(
                    out=s_g[g][:],
                    in_=s_d[g0: g0 + n * P, :].rearrange("(q p) c -> p q c", p=P),
                )
                a_g[g] = io.tile([P, n, CA], dt.bfloat16, name=f"a_g{g}",
                                 tag="a_g", padded_shape=[P, GRP, CA])
                nc.sync.dma_start(
                    out=a_g[g][:],
                    in_=a_d[g0: g0 + n * P, :].rearrange("(q p) c -> p q c", p=P),
                )

            # groups 0 and 1 load before the weights; weights next; rest stream.
            load_group(0)
            load_group(1)
            w_sb = const.tile([P, KC, 2 * CA], dt.bfloat16)
            nc.sync.dma_start(out=w_sb[:, 0, :], in_=w_d[0:P, :])
            load_group(2)
            load_group(3)
            for k in range(1, KC):
                nc.sync.dma_start(out=w_sb[:, k, :], in_=w_d[k * P: (k + 1) * P, :])

            # ---- per-group stat tiles ----
            st6 = [None] * NG
            mv = [None] * NG
            inv_s = [None] * NG
            asum = [None] * NG
            assq = [None] * NG
            mu_a = [None] * NG
            y_a = [None] * NG

            def emit_tile_stats(jj):
                """bn_stats for s, sum/sumsq for a, for global tile jj."""
                g, i = grp_of(jj)
                n = GROUPS[g]
                if st6[g] is None:
                    st6[g] = stat.tile([P, n, 6], dt.float32, name=f"st6_{g}",
                                       tag="st6", padded_shape=[P, GRP, 6])
                    asum[g] = stat.tile([P, n], dt.float32, name=f"asum{g}",
                                        tag="asum", padded_shape=[P, GRP])
                    assq[g] = stat.tile([P, n], dt.float32, name=f"assq{g}",
                                        tag="assq", padded_shape=[P, GRP])
                s_t = s_g[g][:, i, :]
                a_t = a_g[g][:, i, :]
                nc.vector.bn_stats(st6[g][:, i, :], s_t[:])
                ascr = scr.tile([P, CA], dt.bfloat16, name="ascr", tag="ascr")
                nc.scalar.activation(
                    out=ascr[:], in_=a_t[:], func=AF.Identity,
                    accum_out=asum[g][:, i: i + 1],
                )
                sscr = scr.tile([P, CA], dt.bfloat16, name="sscr", tag="sscr")
                nc.scalar.activation(
                    out=sscr[:], in_=a_t[:], func=AF.Square,
                    accum_out=assq[g][:, i: i + 1],
                )

            def newton_rsqrt(dst, ve, n_col):
                """dst = 1/sqrt(ve), ve ~ 1.0. One seeded Newton iteration x2."""
                y0 = dst
                nc.vector.tensor_scalar(
                    out=y0[:], in0=ve[:], scalar1=-0.5, scalar2=1.5,
                    op0=OP.mult, op1=OP.add,
                )
                u = stat.tile([P, n_col], dt.float32, name="newt_u", tag="newt_u",
                              padded_shape=[P, GRP])
                nc.vector.tensor_tensor(out=u[:], in0=y0[:], in1=y0[:], op=OP.mult)
                nc.vector.tensor_tensor(out=u[:], in0=u[:], in1=ve[:], op=OP.mult)
                nc.vector.tensor_scalar(
                    out=u[:], in0=u[:], scalar1=-0.5, scalar2=1.5,
                    op0=OP.mult, op1=OP.add,
                )
                nc.vector.tensor_tensor(out=y0[:], in0=y0[:], in1=u[:], op=OP.mult)

            def emit_group_smalls(g):
                n = GROUPS[g]
                mv[g] = stat.tile([P, n, 2], dt.float32, name=f"mv{g}", tag="mv",
                                  padded_shape=[P, GRP, 2])
                for i in range(n):
                    nc.vector.bn_aggr(mv[g][:, i, :], st6[g][:, i, :])
                # s: inv_s = rsqrt(var + eps)
                inv_s[g] = stat.tile([P, n], dt.float32, name=f"invs{g}", tag="invs",
                                     padded_shape=[P, GRP])
                ve = stat.tile([P, n], dt.float32, name="ve_s", tag="ve_s",
                               padded_shape=[P, GRP])
                nc.vector.tensor_scalar(
                    out=ve[:], in0=mv[g][:, :, 1], scalar1=EPS, scalar2=None,
                    op0=OP.add,
                )
                newton_rsqrt(inv_s[g], ve, n)
                # a: mu_a, var_a from asum/assq; y_a = rsqrt(var + eps)
                mu_a[g] = stat.tile([P, n], dt.float32, name=f"mua{g}", tag="mua",
                                    padded_shape=[P, GRP])
                nc.vector.tensor_scalar(
                    out=mu_a[g][:], in0=asum[g][:], scalar1=1.0 / CA, scalar2=None,
                    op0=OP.mult,
                )
                mu2 = stat.tile([P, n], dt.float32, name="mu2", tag="mu2",
                                padded_shape=[P, GRP])
                nc.vector.tensor_tensor(
                    out=mu2[:], in0=mu_a[g][:], in1=mu_a[g][:], op=OP.mult
                )
                va = stat.tile([P, n], dt.float32, name="va", tag="va",
                               padded_shape=[P, GRP])
                nc.vector.scalar_tensor_tensor(
                    out=va[:], in0=assq[g][:], scalar=1.0 / CA, in1=mu2[:],
                    op0=OP.mult, op1=OP.subtract,
                )
                nc.vector.tensor_scalar(
                    out=va[:], in0=va[:], scalar1=EPS, scalar2=None, op0=OP.add,
                )
                y_a[g] = stat.tile([P, n], dt.float32, name=f"ya{g}", tag="ya",
                                   padded_shape=[P, GRP])
                newton_rsqrt(y_a[g], va, n)

            # ---- prologue stats: tiles 0..3 (groups 0..2) primed upfront ----
            for g in (0, 1, 2):
                for i in range(GROUPS[g]):
                    emit_tile_stats(G_FIRST[g] + i)
                emit_group_smalls(g)

            # ---- s-pipeline stage tiles ----
            s_hat_t = [None] * NTILES
            sT_t = [None] * NTILES
            a_n_t = [None] * NTILES
            g_t = [None] * NTILES
            t2_t = [None] * NTILES
            pgk_t = [None] * NTILES
            psT_t = [None] * NTILES

            def emit_s_stage(j):
                """s_hat + PE transposes + copy to SBUF for tile j."""
                g, i = grp_of(j)
                s_t = s_g[g][:, i, :]
                sh = wp.tile([P, CS], dt.bfloat16, name="s_hat", tag="s_hat")
                nc.vector.tensor_scalar(
                    out=sh[:], in0=s_t[:],
                    scalar1=mv[g][:, i, 0:1], scalar2=inv_s[g][:, i: i + 1],
                    op0=OP.subtract, op1=OP.mult,
                )
                s_hat_t[j] = sh
                psT = pst.tile([P, KC, P], dt.bfloat16, name="psT", tag="psT")
                for k in range(KC):
                    nc.tensor.transpose(
                        psT[:, k, :], sh[:, k * P: (k + 1) * P], id_sb[:]
                    )
                psT_t[j] = psT

            def emit_s_copy(j):
                sT = wp.tile([P, KC, P], dt.bfloat16, name="sT", tag="sT")
                nc.vector.tensor_copy(out=sT[:], in_=psT_t[j][:])
                sT_t[j] = sT

            def emit_mains(j):
                """bias + main matmuls for tile j into a fresh pgk buffer."""
                pgk = pgk_pool.tile([P, 3 * 512], dt.float32, name="pgk", tag="pgk")
                pgk_t[j] = pgk
                sT = sT_t[j]
                # bias: start=True clears banks 0 and 1 entirely (bias row is
                # zero-padded over cols 768:1024)
                nc.tensor.matmul(
                    pgk[:, 0:512], lhsT=on_sb[:, :], rhs=br_sb[:, 0:512],
                    start=True, stop=False, skip_group_check=True,
                )
                nc.tensor.matmul(
                    pgk[:, 512:1024], lhsT=on_sb[:, :], rhs=br_sb[:, 512:1024],
                    start=True, stop=False, skip_group_check=True,
                )
                # mains: 3 N-chunks of 512 per k; cols 0:768 gate, 768:1536 skip
                for k in range(KC):
                    for nn in range(3):
                        nsl = slice(nn * 512, (nn + 1) * 512)
                        nc.tensor.matmul(
                            pgk[:, nsl],
                            lhsT=sT[:, k, :],
                            rhs=w_sb[:, k, nsl],
                            start=(k == 0 and nn == 2),
                            stop=(k == KC - 1),
                            skip_group_check=True,
                        )

            def emit_deferred(j):
                """sigmoid + epilogue + out DMA for tile j."""
                g, i = grp_of(j)
                a_t = a_g[g][:, i, :]
                pgk = pgk_t[j]
                gt = wp.tile([P, CA], dt.bfloat16, name="g", tag="g")
                nc.scalar.activation(out=gt[:], in_=pgk[:, 0:CA], func=AF.Sigmoid)
                g_t[j] = gt
                m = wp.tile([P, CA], dt.bfloat16, name="m", tag="m")
                if j < 16:
                    # A/B experiment: half-width stt ops (fast-mode probe)
                    for h in range(2):
                        hs = slice(h * 384, (h + 1) * 384)
                        nc.vector.scalar_tensor_tensor(
                            out=m[:, hs], in0=a_t[:, hs],
                            scalar=mu_a[g][:, i: i + 1], in1=gt[:, hs],
                            op0=OP.subtract, op1=OP.mult,
                        )
                else:
                    nc.vector.scalar_tensor_tensor(
                        out=m[:], in0=a_t[:],
                        scalar=mu_a[g][:, i: i + 1], in1=gt[:],
                        op0=OP.subtract, op1=OP.mult,
                    )
                o_t = wp.tile([P, CA], dt.bfloat16, name="o_t", tag="o_t")
                nc.vector.scalar_tensor_tensor(
                    out=o_t[:], in0=m[:],
                    scalar=y_a[g][:, i: i + 1], in1=pgk[:, 768:1536],
                    op0=OP.mult, op1=OP.add,
                )
                r0 = j * P
                nc.sync.dma_start(out=out_d[r0: r0 + P, :], in_=o_t[:])

            # ---- prologue priming of the s-pipeline for tile 0 ----
            emit_s_stage(0)
            emit_s_copy(0)

            # ---- main loop ----
            STAT_LEAD = 4  # stats for tile j+4 are emitted during tile j
            for g in range(NG):
                if g >= 2 and g + 2 < NG:
                    load_group(g + 2)
                for i in range(GROUPS[g]):
                    j = G_FIRST[g] + i
                    # ---- phase B for tile j ----
                    if j > 0:
                        emit_deferred(j - 1)
                    # trickled stats with a flat 4-tile lead
                    jj = j + STAT_LEAD
                    if jj < NTILES:
                        emit_tile_stats(jj)
                        gj, ij = grp_of(jj)
                        if ij == GROUPS[gj] - 1:
                            emit_group_smalls(gj)
                    if j + 1 < NTILES:
                        emit_s_stage(j + 1)
                    emit_mains(j)
                    if j + 1 < NTILES:
                        emit_s_copy(j + 1)

            emit_deferred(NTILES - 1)

    nc.compile()
    return nc


def _get_graph():
    if "nc" not in _BUILD_CACHE:
        _BUILD_CACHE["nc"] = _build_graph()
    return _BUILD_CACHE["nc"]


def _host_prep(a, s, ln_s_w, W_s, b_s, W_nb):
    """Shard inputs and prepare derived weights."""
    bf16 = ml_dtypes.bfloat16
    a2 = np.ascontiguousarray(a.reshape(TOK, CA)).astype(bf16)
    s2 = np.ascontiguousarray(s.reshape(TOK, CS)).astype(bf16)

    wg = (W_s * ln_s_w[None, :]).astype(np.float32)      # [CA, CS]
    wk = (W_nb * ln_s_w[None, :]).astype(np.float32)     # [CA, CS]
    wcat = np.concatenate([wg, wk], axis=0)              # [2CA, CS]
    wcatT = np.ascontiguousarray(wcat.T).astype(bf16)    # [CS, 2CA]
    brow = np.zeros((1, 1024), dtype=bf16)
    brow[0, :CA] = b_s.astype(np.float32).astype(bf16)
    ones1 = np.ones((1, P), dtype=bf16)
    ident = np.eye(P, dtype=bf16)

    in_maps = []
    for c in range(NCORES):
        in_maps.append(
            {
                "a": np.ascontiguousarray(a2[c * TPC: (c + 1) * TPC]),
                "s": np.ascontiguousarray(s2[c * TPC: (c + 1) * TPC]),
                "wcat": wcatT,
                "brow": brow,
                "ones1": ones1,
                "ident": ident,
            }
        )
    return in_maps


def _install_ntff_hook():
    """Register the axon NTFF profile hook that the container's antenv stub lacks."""
    import types
    import antenv

    if "antenv.axon_hooks" not in sys.modules:
        mod = types.ModuleType("antenv.axon_hooks")
        mod._hook = None

        def set_axon_ntff_profile_hook(h):
            mod._hook = h

        def get_axon_ntff_profile_hook():
            return mod._hook

        mod.set_axon_ntff_profile_hook = set_axon_ntff_profile_hook
        mod.get_axon_ntff_profile_hook = get_axon_ntff_profile_hook
        sys.modules["antenv.axon_hooks"] = mod
        antenv.axon_hooks = mod

    hooks = sys.modules["antenv.axon_hooks"]
    if hooks._hook is None:
        from trn_agent_boot.trn_boot import _ntff_profile_via_ctypes

        hooks.set_axon_ntff_profile_hook(
            _ntff_profile_via_ctypes("/opt/axon/libaxon_pjrt.so")
        )

    # upload_artifacts needs external bucket access; stub it out.
    from concourse import bass_utils

    bass_utils.upload_artifacts = lambda tmpdir: f"local:{tmpdir}"


def run(inputs, trace=False):
    """Run on 8 NeuronCores. Returns (out_full [B,N,CA] f32, exec_time_ns|None)."""
    from concourse.bass_utils import run_bass_kernel_spmd

    if trace:
        _install_ntff_hook()
    nc = _get_graph()
    in_maps = _host_prep(**inputs)
    res = run_bass_kernel_spmd(
        nc, in_maps, core_ids=list(range(NCORES)), trace=trace
    )
    outs = [np.asarray(res.results[c]["out"], dtype=np.float32) for c in range(NCORES)]
    full = np.concatenate(outs, axis=0).reshape(B, N, CA)
    return full, res.exec_time_ns


def kernel(**inputs):
    out, _ = run(inputs, trace=False)
    return out
